# revision 31
# baseline (speedup 1.0000x reference)
"""Trainium2 Bass kernel for nn_EnhancedSelfAttention (N=8, S=2048, D=1024).

Strategy: data-parallel over batch N across the 8 NeuronCores (one batch
element per core). The only cross-batch dependency (max over batch) is folded
into host-side input marshalling; each core runs an independent fused kernel.

Per-core pipeline (phases in emission order):
  A: zT = x W2^T (PSUM), rT = relu(zT+b2) bf16, lT = relu(mbT-zT+b2) bf16
     where mbT = (xmax W2^T)^T is batch-invariant, computed on host.
  E: hT = relu(x W1^T + b1) bf16
  B: S2T[j,i] = r_j.l_i (PSUM, transposed orientation so the bi-softmax
     normalization axis lands on the PSUM partition dim); VE row-max m_j,
     ACT exp((s-m)/32 + ln224) -> E2T bf16 (shifted, <=224); VE row-sum
     -> denominators.
  C: PE-transpose all E2T tiles -> Q fp8e4 [i-part, j-free] (the fp8 cast
     happens on the PSUM->SBUF copies, split between ACT and DVE).
  D: O2 = Q^T V8 via fp8 DoubleRow matmuls (2x PE throughput). V8 is the
     host-prepared fp8 operand (1-coeff)*(lrs - g) with g the per-feature
     midrange of lrs: the softmax weights sum to 1 over the contraction, so
     g can be subtracted from V and (1-coeff)*g added back at the blend --
     halving the fp8 quantization error of V. Normalization by the bf16
     denominators cancels the per-column exp shift exactly.
  F: S1 upper triangle only (symmetric), exp -> E1 bf16; lower triangle
     filled by PE-transposes one row behind the exps; row-sums via plain DVE
     free-dim reduces over the completed rows (no PE piggyback matmuls).
  G: O1 = E1 xnc (xnc = coeff*x in bf16, folding the blend coefficient);
     final = O1/rowsum + O2 + (1-coeff)*g, streamed out per block.

All bf16 matmuls with fp32 PSUM accumulation; the O2 matmul is fp8e4
DoubleRow (both operands fp8). Softmax shifts for E1 are skipped: logits lie
in [7, 58]*32 so exp stays in fp32/bf16 range and the softmax is
mathematically identical to the max-subtracted reference.
"""

import math
import sys

sys.path.insert(0, "/opt/trn_rl_repo")

import numpy as np
import ml_dtypes

import concourse.bass as bass
import concourse.tile as tile
from concourse import mybir
from concourse.vector_clock import ScopedClock

BF = mybir.dt.bfloat16
F32 = mybir.dt.float32
FP8 = mybir.dt.float8e4
N, S, D = 8, 2048, 1024
ST, DT, ET = S // 128, D // 128, D // 128  # 16, 8, 8
INV_SCALE = 1.0 / 32.0  # 1/sqrt(D)
LN224 = math.log(224.0)
NCHUNK = 512  # matmul moving free dim (one PSUM bank of fp32)

MAX_WAITS = 1  # walrus codegen in this image rejects instructions with more


def _patch_tile_drain():
    """walrus in this image rejects >MAX_WAITS sem waits on one instruction;
    spread excess waits onto preceding same-engine nops (both for the
    end-of-context drain and for every scheduled instruction)."""
    import concourse.tile as tile_mod

    if getattr(tile_mod.TileContext, "_waitsplit_patched", False):
        return

    _orig_lower = tile_mod.TileContext._lower_ordered_insts
    _ctr = [0]

    def _lower_split(self, ordered):
        for bb, insts in ordered.items():
            out = []
            for inst in insts:
                si = getattr(inst, "sync_info", None)
                if si is not None and len(si.on_wait) > MAX_WAITS:
                    waits = list(si.on_wait)
                    keep = waits[-MAX_WAITS:]
                    extra = waits[:-MAX_WAITS]
                    for i in range(0, len(extra), MAX_WAITS):
                        _ctr[0] += 1
                        n = mybir.InstNoOp(
                            name=f"waitsplit_{_ctr[0]}",
                            engine=inst.engine,
                            ins=[],
                            outs=[],
                            sync_info=mybir.SyncInfo(
                                on_wait=extra[i : i + MAX_WAITS], on_update=[]
                            ),
                        )
                        out.append(n)
                    inst.sync_info = mybir.SyncInfo(
                        on_wait=keep, on_update=list(si.on_update)
                    )
                out.append(inst)
            insts[:] = out
        return _orig_lower(self, ordered)

    tile_mod.TileContext._lower_ordered_insts = _lower_split

    def _drain_and_barrier_split(self, tick_clock, wait_clock):
        nc = self.nc
        probe = nc.sync.nop(nofuse=True, hint="drain_waits")
        wait_clock.add_sem_waits(probe.ins, ScopedClock({None: tick_clock.global_clock}))
        si = probe.ins.sync_info
        waits = list(si.on_wait) if si is not None else []
        if len(waits) > MAX_WAITS:
            probe.ins.sync_info = mybir.SyncInfo(
                on_wait=waits[:MAX_WAITS], on_update=list(si.on_update)
            )
            rest = waits[MAX_WAITS:]
            for i in range(0, len(rest), MAX_WAITS):
                n = nc.sync.nop(nofuse=True, hint="drain_waits")
                n.ins.sync_info = mybir.SyncInfo(
                    on_wait=rest[i : i + MAX_WAITS], on_update=[]
                )
        nc.sync.drain()
        nc.all_engine_barrier()
        assert self.sems is not None
        popped = nc._tile_sem_poison_stack.pop()
        assert popped is self._sem_poison
        nc.clear_and_free_semaphores(list(self.sems.allocated().values()))
        nc.all_engine_barrier()

    tile_mod.TileContext._drain_and_barrier = _drain_and_barrier_split
    tile_mod.TileContext._waitsplit_patched = True


_patch_tile_drain()


def _emit(tc, io):
    nc = tc.nc
    Relu = mybir.ActivationFunctionType.Relu
    Exp = mybir.ActivationFunctionType.Exp
    Copy = mybir.ActivationFunctionType.Copy
    Mult = mybir.AluOpType.mult
    Add = mybir.AluOpType.add
    Max = mybir.AluOpType.max
    DR = mybir.MatmulPerfMode.DoubleRow
    AxX = mybir.AxisListType.X

    small = tc.alloc_tile_pool(name="small", bufs=1, side="left")
    b1_sb = small.tile([128, ET], F32, tag="b1")
    b2_sb = small.tile([128, ET], F32, tag="b2")
    gomc_sb = small.tile([128, D], F32, tag="gomc")
    ident_sb = small.tile([128, 128], BF, tag="ident")
    dsum_sb = small.tile([128, ST], F32, tag="dsum")
    csinv_sb = small.tile([128, ST], F32, tag="csinv")
    bias_sb = small.tile([128, ST], F32, tag="bias")
    m_sb = small.tile([128, ST], F32, tag="m")
    mx4_sb = small.tile([128, ST, 4], F32, tag="mx4")
    rs_sb = small.tile([128, ST], F32, tag="rs")
    rinv_sb = small.tile([128, ST], F32, tag="rinv")

    # pools (allocation order is chosen so each side's pool lifetimes nest)
    xTp = tc.alloc_tile_pool(name="xTp", bufs=1, side="left")
    xT_sb = xTp.tile([128, DT, S], BF, tag="xT")
    rlp = tc.alloc_tile_pool(name="rlp", bufs=1, side="left")
    rT_sb = rlp.tile([128, ET, S], BF, tag="rT")
    lT_sb = rlp.tile([128, ET, S], BF, tag="lT")
    hTp = tc.alloc_tile_pool(name="hTp", bufs=1, side="right")
    hT_sb = hTp.tile([128, ET, S], BF, tag="hT")
    w1p = tc.alloc_tile_pool(name="w1p", bufs=1, side="right")
    w1T_sb = w1p.tile([128, DT, D], BF, tag="w1T")
    mbp = tc.alloc_tile_pool(name="mbp", bufs=1, side="right")
    mbT_sb = mbp.tile([128, ET, S], BF, tag="mbT")
    w2p = tc.alloc_tile_pool(name="w2p", bufs=1, side="right")
    w2T_sb = w2p.tile([128, DT, D], BF, tag="w2T")

    # ---- startup DMAs. DMA bandwidth is shared round-robin across all
    # outstanding transfers and there is a ~10us DMA-start ramp, so only the
    # phase-A inputs (xT, w2T) are issued at t0 (split across both DMA-capable
    # queues); mbT/w1T triggers are paced onto the in-order ACT queue behind
    # phase-A relus so their transfers start after the critical xT window.
    for dt in range(DT):
        eng = nc.sync if dt % 2 == 0 else nc.gpsimd
        eng.dma_start(out=xT_sb[:, dt, :], in_=io["xT"][dt * 128 : (dt + 1) * 128, :])
    for dt in range(DT):
        eng = nc.gpsimd if dt % 2 == 0 else nc.sync
        eng.dma_start(out=w2T_sb[:, dt, :], in_=io["w2T"][dt * 128 : (dt + 1) * 128, :])
    nc.scalar.dma_start(out=mbT_sb[:, 0, :], in_=io["mbT"][0:128, :])
    nc.scalar.dma_start(out=b2_sb, in_=io["b2"].rearrange("(t p) -> p t", p=128))
    nc.scalar.dma_start(out=b1_sb, in_=io["b1"].rearrange("(t p) -> p t", p=128))
    nc.scalar.dma_start(
        out=gomc_sb,
        in_=bass.AP(tensor=io["gomc"].tensor, offset=io["gomc"].offset, ap=[[0, 128], [1, D]]),
    )
    nc.scalar.dma_start(out=ident_sb, in_=io["ident"][:, :])

    # ---------------- A: zT -> rT, lT --------------------------------------
    # two single-buffer PSUM pools (even/odd groups) instead of one bufs=2
    # pool: phase C's transpose pool can then allocate from the even pool's
    # banks as soon as the second-to-last B group drains, instead of waiting
    # for the last group's rowmax+exp chain
    psAe = tc.alloc_tile_pool(name="psAe", bufs=1, space="PSUM", side="left")
    psAo = tc.alloc_tile_pool(name="psAo", bufs=1, space="PSUM", side="right")
    subp = tc.alloc_tile_pool(name="subp", bufs=2, side="left")
    for et in range(ET):
        pool = psAe if et % 2 == 0 else psAo
        ps = pool.tile([128, S], F32, tag="ps_mm", name=f"psA_{et}")
        for dt in range(DT):
            lhsT = w2T_sb[:, dt, et * 128 : (et + 1) * 128]
            for c in range(S // NCHUNK):
                nc.tensor.matmul(
                    ps[:, c * NCHUNK : (c + 1) * NCHUNK],
                    lhsT,
                    xT_sb[:, dt, c * NCHUNK : (c + 1) * NCHUNK],
                    start=(dt == 0),
                    stop=(dt == DT - 1),
                )
        nc.scalar.activation(rT_sb[:, et, :], ps, Relu, bias=b2_sb[:, et : et + 1])
        # paced prefetch: fires behind this et's relu on the in-order ACT
        # queue, after the critical startup loads, without delaying the relu
        if et + 1 < ET:
            nc.scalar.dma_start(
                out=mbT_sb[:, et + 1, :], in_=io["mbT"][(et + 1) * 128 : (et + 2) * 128, :]
            )
        if et in (4, 5):
            half = et - 4
            nc.scalar.dma_start(
                out=w1T_sb[:, 4 * half : 4 * half + 4, :],
                in_=io["w1T"].rearrange("(t p) e -> p t e", p=128)[:, 4 * half : 4 * half + 4, :],
            )
        for h in range(2):
            hs = slice(h * (S // 2), (h + 1) * (S // 2))
            sub = subp.tile([128, S // 2], F32, tag="sub", name=f"sub_{et}_{h}")
            nc.vector.tensor_sub(sub, mbT_sb[:, et, hs], ps[:, hs])
            nc.vector.tensor_scalar(lT_sb[:, et, hs], sub, b2_sb[:, et : et + 1], 0.0, Add, Max)
    subp.release()

    # ---------------- E: hT = relu(x W1^T + b1) ----------------------------
    w2p.release()
    mbp.release()
    for et in range(ET):
        pool = psAe if et % 2 == 0 else psAo
        ps = pool.tile([128, S], F32, tag="ps_mm", name=f"psE_{et}")
        for dt in range(DT):
            lhsT = w1T_sb[:, dt, et * 128 : (et + 1) * 128]
            for c in range(S // NCHUNK):
                nc.tensor.matmul(
                    ps[:, c * NCHUNK : (c + 1) * NCHUNK],
                    lhsT,
                    xT_sb[:, dt, c * NCHUNK : (c + 1) * NCHUNK],
                    start=(dt == 0),
                    stop=(dt == DT - 1),
                )
        nc.scalar.activation(hT_sb[:, et, :], ps, Relu, bias=b1_sb[:, et : et + 1])
    w1p.release()

    # ---------------- B: S2T -> E2T (shifted, bf16) + denominators ---------
    # chunk-major so the per-chunk row-maxes overlap the matmuls; the post-
    # group serial chain (combine+bias+exp) shrinks to ~2.5us, which is what
    # gates the PSUM-pool handoff to the transposes of phase C
    E2p = tc.alloc_tile_pool(name="E2p", bufs=1, side="right")
    E2T_sb = E2p.tile([128, ST, S], BF, tag="E2T")
    for jt in range(ST):
        pool = psAe if jt % 2 == 0 else psAo
        ps = pool.tile([128, S], F32, tag="ps_mm", name=f"psB_{jt}")
        for et in range(ET):
            lhsT = rT_sb[:, et, jt * 128 : (jt + 1) * 128]
            for c in range(S // NCHUNK):
                sl = slice(c * NCHUNK, (c + 1) * NCHUNK)
                nc.tensor.matmul(
                    ps[:, sl],
                    lhsT,
                    lT_sb[:, et, sl],
                    start=(et == 0),
                    stop=(et == ET - 1),
                )
        nc.vector.tensor_reduce(m_sb[:, jt : jt + 1], ps, AxX, mybir.AluOpType.max)
        nc.vector.tensor_scalar(
            bias_sb[:, jt : jt + 1], m_sb[:, jt : jt + 1], -INV_SCALE, LN224, Mult, Add
        )
        nc.scalar.activation(
            E2T_sb[:, jt, :], ps, Exp, bias=bias_sb[:, jt : jt + 1], scale=INV_SCALE
        )
    # denominators: deferred so the in-order VE queue never blocks on an exp
    # mid-B; they run during the C transposes, well before D needs csinv
    for jt in range(ST):
        nc.vector.tensor_reduce(dsum_sb[:, jt : jt + 1], E2T_sb[:, jt, :], AxX, Add)
    nc.vector.reciprocal(csinv_sb, dsum_sb)
    rlp.release()
    xTp.release()

    # ---------------- C+D interleaved: transposes feed fp8 DoubleRow O2 ----
    O2p = tc.alloc_tile_pool(name="O2p", bufs=1, side="left")
    O2_sb = O2p.tile([128, ST, D], BF, tag="O2")
    v8p = tc.alloc_tile_pool(name="v8p", bufs=1, side="right")
    v8_sb = v8p.tile([128, ST, D], FP8, tag="v8")
    nc.gpsimd.dma_start(out=v8_sb, in_=io["v8"].rearrange("(t p) d -> p t d", p=128))
    q8p = tc.alloc_tile_pool(name="q8p", bufs=1, side="right")
    q8_sb = q8p.tile([128, ST, S], FP8, tag="q8")
    psAe.release()
    psC = tc.alloc_tile_pool(name="psC", bufs=4, space="PSUM", side="left")
    psAo.release()
    psD = tc.alloc_tile_pool(name="psD", bufs=2, space="PSUM", side="right")

    def c_group(g):
        # transpose the 16 tiles of column-group g (j-blocks 4g..4g+3) into
        # Q[p=i, f=j]; fp8 cast happens on the PSUM->SBUF copies (ACT/DVE)
        for it in range(ST):
            pt = psC.tile([128, 512], BF, tag="pt", name=f"pt_{it}_{g}")
            for q in range(4):
                jt = 4 * g + q
                nc.tensor.transpose(
                    pt[:, q * 128 : (q + 1) * 128],
                    E2T_sb[:, jt, it * 128 : (it + 1) * 128],
                    ident_sb,
                )
            dst = q8_sb[:, it, g * 512 : (g + 1) * 512]
            if it % 2 == 0:
                nc.scalar.activation(dst, pt, Copy)
            else:
                nc.vector.tensor_copy(dst, pt)

    def d_group(g):
        for jt in range(4 * g, 4 * g + 4):
            ps = psD.tile([128, D], F32, tag="ps_o2", name=f"o2_{jt}")
            for ip in range(ST // 2):
                lhsT = q8_sb[:, 2 * ip : 2 * ip + 2, jt * 128 : (jt + 1) * 128]
                st_, sp_ = (ip == 0), (ip == ST // 2 - 1)
                nc.tensor.matmul(
                    ps[:, 0:NCHUNK],
                    lhsT,
                    v8_sb[:, 2 * ip : 2 * ip + 2, 0:NCHUNK],
                    start=st_,
                    stop=sp_,
                    perf_mode=DR,
                )
                nc.tensor.matmul(
                    ps[:, NCHUNK:D],
                    lhsT,
                    v8_sb[:, 2 * ip : 2 * ip + 2, NCHUNK:D],
                    start=st_,
                    stop=sp_,
                    perf_mode=DR,
                )
            nc.scalar.activation(O2_sb[:, jt, :], ps, Copy, scale=csinv_sb[:, jt : jt + 1])

    c_group(0)
    c_group(1)
    d_group(0)
    c_group(2)
    d_group(1)
    c_group(3)
    d_group(2)
    d_group(3)
    psD.release()
    psC.release()
    q8p.release()
    v8p.release()
    E2p.release()

    # ---------------- F: S1 upper -> E1 bf16 + lower fills -----------------
    xnp = tc.alloc_tile_pool(name="xnp", bufs=1, side="left")
    xn_sb = xnp.tile([128, ST, D], BF, tag="xn")
    nc.gpsimd.dma_start(out=xn_sb, in_=io["xnc"].rearrange("(t p) d -> p t d", p=128))
    E1p = tc.alloc_tile_pool(name="E1p", bufs=1, side="left")
    E1_sb = E1p.tile([128, ST, S], BF, tag="E1")
    psB = tc.alloc_tile_pool(name="psB", bufs=2, space="PSUM")
    psF = tc.alloc_tile_pool(name="psF", bufs=4, space="PSUM")
    H = 1024

    def fill_col(k):
        # fill lower-triangle column k from row k (exp'd one iteration ago so
        # the transposes never head-block the PE queue); copies split ACT/DVE
        for jt in range(k + 1, ST):
            pt = psF.tile([128, 128], BF, tag="ptf", name=f"ptf_{jt}_{k}")
            nc.tensor.transpose(pt, E1_sb[:, k, jt * 128 : (jt + 1) * 128], ident_sb)
            dst = E1_sb[:, jt, k * 128 : (k + 1) * 128]
            if jt % 2 == 0:
                nc.scalar.activation(dst, pt, Copy)
            else:
                nc.vector.tensor_copy(dst, pt)

    for it in range(ST):
        c0 = it * 128
        halves = []
        for h in range(2):
            lo, hi = max(c0, h * H), (h + 1) * H
            if lo < hi:
                halves.append(
                    (h, lo, hi, psB.tile([128, H], F32, tag="ps_s1", name=f"s1_{it}_{h}"))
                )
        for et in range(ET):
            lhsT = hT_sb[:, et, c0 : c0 + 128]
            for h, lo, hi, psh in halves:
                c = lo
                while c < hi:
                    nxt = min((c // NCHUNK + 1) * NCHUNK, hi)
                    nc.tensor.matmul(
                        psh[:, c - h * H : nxt - h * H],
                        lhsT,
                        hT_sb[:, et, c:nxt],
                        start=(et == 0),
                        stop=(et == ET - 1),
                    )
                    c = nxt
        if it >= 1:
            fill_col(it - 1)
        for h, lo, hi, psh in halves:
            nc.scalar.activation(
                E1_sb[:, it, lo:hi], psh[:, lo - h * H : hi - h * H], Exp, scale=INV_SCALE
            )
        if it >= 2:
            # row it-2 is complete (upper exp'd at it-2, last fill at it-1):
            # row-sum = plain VE free-dim reduce, no PE piggyback needed
            nc.vector.tensor_reduce(rs_sb[:, it - 2 : it - 1], E1_sb[:, it - 2, :], AxX, Add)
    for it in range(ST - 2, ST):
        nc.vector.tensor_reduce(rs_sb[:, it : it + 1], E1_sb[:, it, :], AxX, Add)
    nc.vector.reciprocal(rinv_sb, rs_sb)
    hTp.release()
    psF.release()
    psB.release()

    # ---------------- G: O1 + blend + out ----------------------------------
    blend = tc.alloc_tile_pool(name="blend", bufs=3, side="right")
    psG = tc.alloc_tile_pool(name="psG", bufs=2, space="PSUM")
    for it in range(ST):
        last = it == ST - 1
        if not last:
            ps = psG.tile([128, D], F32, tag="ps_o1")
            for jt in range(ST):
                lhsT = E1_sb[:, jt, it * 128 : (it + 1) * 128]
                st_, sp_ = (jt == 0), (jt == ST - 1)
                for a, b in [(0, NCHUNK), (NCHUNK, D)]:
                    nc.tensor.matmul(ps[:, a:b], lhsT, xn_sb[:, jt, a:b], start=st_, stop=sp_)
            pieces = [(0, D, ps)]
        else:
            # tail: per-chunk psum tiles so each blend+DMA starts as soon as
            # its own matmuls finish; shrinking chunks minimize the exposed
            # final chain
            pieces = []
            for a, b in [(0, 384), (384, 768), (768, 896), (896, 1024)]:
                psq = psG.tile([128, b - a], F32, tag="ps_o1t", name=f"o1t_{a}")
                for jt in range(ST):
                    lhsT = E1_sb[:, jt, it * 128 : (it + 1) * 128]
                    nc.tensor.matmul(
                        psq, lhsT, xn_sb[:, jt, a:b], start=(jt == 0), stop=(jt == ST - 1)
                    )
                pieces.append((a, b, psq))
        for a, b, ps in pieces:
            o1 = blend.tile([128, b - a], F32, tag="o1", name=f"o1_{it}_{a}")
            f1 = blend.tile([128, b - a], F32, tag="f1", name=f"f1_{it}_{a}")
            fin = blend.tile([128, b - a], F32, tag="fin", name=f"fin_{it}_{a}")
            nc.vector.tensor_scalar_mul(o1, ps[:, 0 : b - a], rinv_sb[:, it : it + 1])
            nc.vector.tensor_add(f1, o1, O2_sb[:, it, a:b])
            nc.vector.tensor_add(fin, f1, gomc_sb[:, a:b])
            nq = (b - a + NCHUNK - 1) // NCHUNK
            for q in range(nq):
                sl = slice(q * NCHUNK, min((q + 1) * NCHUNK, b - a))
                eng = nc.sync if (last or (a // NCHUNK + q) % 2 == 0) else nc.gpsimd
                eng.dma_start(
                    out=io["fin"][it * 128 : (it + 1) * 128, a + sl.start : a + sl.stop],
                    in_=fin[:, sl],
                )

    for p in (blend, psG, E1p, xnp, O2p, small):
        p.release()


def build_bass():
    nc = bass.Bass("TRN2", target_bir_lowering=False, debug=False)
    io = {}
    for name, shape, dt in [
        ("xT", [D, S], BF),
        ("xnc", [S, D], BF),
        ("mbT", [D, S], BF),
        ("v8", [S, D], FP8),
        ("w1T", [D, D], BF),
        ("w2T", [D, D], BF),
        ("b1", [D], F32),
        ("b2", [D], F32),
        ("gomc", [D], F32),
        ("ident", [128, 128], BF),
    ]:
        io[name] = nc.dram_tensor(name, shape, dt, kind="ExternalInput").ap()
    io["fin"] = nc.dram_tensor("fin", [S, D], F32, kind="ExternalOutput").ap()
    with tile.TileContext(nc) as tc:
        _emit(tc, io)
    return nc


def kernel(x, W1, b1, W2, b2, coeff):
    from concourse.bass_utils import run_bass_kernel_spmd

    x = np.asarray(x, dtype=np.float32)
    W1 = np.asarray(W1, dtype=np.float32)
    W2 = np.asarray(W2, dtype=np.float32)
    b1 = np.asarray(b1, dtype=np.float32)
    b2 = np.asarray(b2, dtype=np.float32)
    coeff = np.asarray(coeff, dtype=np.float32)

    bf16 = ml_dtypes.bfloat16
    fp8 = ml_dtypes.float8_e4m3
    x_max = x.max(axis=0)  # host all-reduce(max) over batch
    mb = x_max @ W2.T  # batch-invariant: (xmax - x) W2^T = mb - x W2^T
    mbT = np.ascontiguousarray(mb.T).astype(bf16)
    w1T = np.ascontiguousarray(W1.T).astype(bf16)
    w2T = np.ascontiguousarray(W2.T).astype(bf16)
    ident = np.eye(128, dtype=bf16)
    omc = 1.0 - coeff
    nc = build_bass()
    in_maps = []
    for b in range(N):
        lr = x_max - x[b]
        g = 0.5 * (lr.min(axis=0) + lr.max(axis=0))
        in_maps.append(
            {
                "xT": np.ascontiguousarray(x[b].T).astype(bf16),
                "xnc": (coeff * x[b]).astype(bf16),
                "mbT": mbT,
                "v8": (omc * (lr - g)).astype(fp8),
                "w1T": w1T,
                "w2T": w2T,
                "b1": b1,
                "b2": b2,
                "gomc": (omc * g).astype(np.float32),
                "ident": ident,
            }
        )
    res = run_bass_kernel_spmd(nc, in_maps, core_ids=list(range(N)))
    out = np.empty((N, S, 2 * D), dtype=np.float32)
    for b in range(N):
        out[b, :, :D] = x[b]
        out[b, :, D:] = res.results[b]["fin"]
    return out
